# revision 10
# baseline (speedup 1.0000x reference)
"""Causal self-attention with RoPE on 8 Trainium2 NeuronCores.

Problem: B=4, T=2048, C=1024, H=16 heads, D=64.
    qkv = x @ W_qkv.T ; q,k = RoPE(q,k) ; att = softmax(causal(q k^T / 8)) ;
    y = att @ v ; out = y @ W_proj.T

Sharding: tensor-parallel over heads -- 2 heads per core. Each core computes
q/k/v for its 2 heads over all tokens, runs causal attention, and produces a
partial projection output (row-parallel W_proj); the host sums the 8 partials
(the TP all-reduce done at gather time).

Per-core pipeline, one batch at a time (tiles double-buffered across batches):
  P1  qkvT in [chan, tok] layout from host-pretransposed xT (fp32r matmuls),
      RoPE fused into the PSUM eviction (shift via SBUF-SBUF partition DMAs).
  P2  V transposed on the PE into V' chunks [s=128, 65] with a shared ones
      column so the AV matmul also produces the softmax denominator.
  P3  S^T tiles [s=128, t<=512] = kT.T @ qT (K=64; h0 on partitions 0-63, h1 on
      64-127 -> concurrent PE row groups). Softmax without max-subtraction
      (logits bounded); exp on ScalarE straight out of PSUM; causal mask only
      on the diagonal 128x128 triangle; fully-masked columns never computed.
  P4  reciprocal (+1 Newton step) of the denominators, partition-broadcast via
      a K=2 selector matmul, normalization fused into yT; projection with
      K=128 (both heads) and partial-output DMA.
"""
import sys

if "/opt/trn_rl_repo" not in sys.path:
    sys.path.insert(0, "/opt/trn_rl_repo")

import numpy as np
from contextlib import ExitStack

import concourse.bass as bass
import concourse.mybir as mybir
import concourse.tile as tile
from concourse import bacc

F32 = mybir.dt.float32
F32R = mybir.dt.float32r
AF = mybir.ActivationFunctionType

B, T, C = 4, 2048, 1024
D = 64
N_CORES = 8
NT = B * T          # 8192 tokens across batches
KCH = C // 128      # 8 contraction chunks
TJ = T // 512       # 4 t-tiles per batch
SI = T // 128       # 16 s-blocks per batch


def build_nc():
    nc = bacc.Bacc("TRN2", target_bir_lowering=False, debug=False)

    xT = nc.dram_tensor("xT", [C, NT], F32R, kind="ExternalInput")
    wq = nc.dram_tensor("wq", [C, 128], F32R, kind="ExternalInput")
    wk = nc.dram_tensor("wk", [C, 128], F32R, kind="ExternalInput")
    wv = nc.dram_tensor("wv", [C, 128], F32R, kind="ExternalInput")
    wp = nc.dram_tensor("wp", [128, C], F32R, kind="ExternalInput")
    cos2 = nc.dram_tensor("cos2", [128, T], F32R, kind="ExternalInput")
    sin2 = nc.dram_tensor("sin2", [128, T], F32R, kind="ExternalInput")
    mask = nc.dram_tensor("mask", [128, 128], F32R, kind="ExternalInput")
    ident = nc.dram_tensor("ident", [128, 128], F32, kind="ExternalInput")
    sel = nc.dram_tensor("sel", [2, 128], F32R, kind="ExternalInput")
    ones = nc.dram_tensor("ones", [128, 1], F32R, kind="ExternalInput")

    out = nc.dram_tensor("out_part", [NT, C], F32, kind="ExternalOutput")

    with tile.TileContext(nc) as tc, ExitStack() as ctx:
        glob = ctx.enter_context(tc.tile_pool(name="glob", bufs=1))
        rot = ctx.enter_context(tc.tile_pool(name="rot", bufs=2))
        work = ctx.enter_context(tc.tile_pool(name="work", bufs=3))
        xw = ctx.enter_context(tc.tile_pool(name="xw", bufs=2))
        psum = ctx.enter_context(tc.tile_pool(name="psum", bufs=1, space="PSUM"))

        # ---- persistent SBUF ----
        wq_sb = glob.tile([128, KCH * 128], F32R, tag="wq")
        wk_sb = glob.tile([128, KCH * 128], F32R, tag="wk")
        wv_sb = glob.tile([128, KCH * 128], F32R, tag="wv")
        wp_sb = glob.tile([128, C], F32R, tag="wp")
        cos_sb = glob.tile([128, T], F32R, tag="cos")
        sin_sb = glob.tile([128, T], F32R, tag="sin")

        mask_sb = glob.tile([128, 128], F32R, tag="mask")
        ident_sb = glob.tile([128, 128], F32, tag="ident")
        sel_sb = glob.tile([2, 128], F32R, tag="sel")
        ones_sb = glob.tile([128, 1], F32R, tag="ones")

        for k in range(KCH):
            nc.sync.dma_start(wq_sb[:, k * 128:(k + 1) * 128], wq[k * 128:(k + 1) * 128, :])
            nc.sync.dma_start(wk_sb[:, k * 128:(k + 1) * 128], wk[k * 128:(k + 1) * 128, :])
            nc.sync.dma_start(wv_sb[:, k * 128:(k + 1) * 128], wv[k * 128:(k + 1) * 128, :])
        nc.sync.dma_start(wp_sb[:], wp[:])
        nc.sync.dma_start(cos_sb[:], cos2[:])
        nc.sync.dma_start(sin_sb[:], sin2[:])
        nc.sync.dma_start(mask_sb[:], mask[:])
        nc.sync.dma_start(ident_sb[:], ident[:])
        nc.sync.dma_start(sel_sb[:], sel[:])
        nc.sync.dma_start(ones_sb[:], ones[:])

        for b in range(B):
            # per-batch rotating tiles (double-buffered across batches)
            q2 = rot.tile([128, T], F32R, tag="q2", name=f"q2_{b}")
            k2 = rot.tile([128, T], F32R, tag="k2", name=f"k2_{b}")
            v2 = rot.tile([128, T], F32, tag="v2", name=f"v2_{b}")
            vp = rot.tile([128, SI * 130], F32R, tag="vp", name=f"vp_{b}")
            yt = rot.tile([128, T], F32R, tag="yt", name=f"yt_{b}")
            rn = rot.tile([33, 2 * T], F32R, tag="rn", name=f"rn_{b}", bufs=1)

            # ones columns of V' (cols 64 and 129 of each 130-wide chunk)
            for i in range(SI):
                nc.sync.dma_start(vp[:, i * 130 + 64:i * 130 + 65], ones_sb[:])
                nc.sync.dma_start(vp[:, i * 130 + 129:i * 130 + 130], ones_sb[:])

            # ---- P1: qkv projection + RoPE ----
            for j in range(TJ):
                t0 = j * 512           # within-batch token offset
                g0 = b * T + t0        # global token offset
                xks = []
                for g in range(2):
                    xk = xw.tile([128, 512 * KCH // 2], F32R, tag="x",
                                 name=f"xk_{b}_{j}_{g}")
                    for kk in range(KCH // 2):
                        k = g * (KCH // 2) + kk
                        nc.sync.dma_start(
                            xk[:, kk * 512:(kk + 1) * 512],
                            xT[k * 128:(k + 1) * 128, g0:g0 + 512])
                    xks.append(xk)

                for w_sb, kind in ((wq_sb, "q"), (wk_sb, "k"), (wv_sb, "v")):
                    ps = psum.tile([128, 512], F32, tag="qkv", bufs=2,
                                   name=f"ps_{kind}_{b}_{j}")
                    for k in range(KCH):
                        nc.tensor.matmul(
                            ps[:], w_sb[:, k * 128:(k + 1) * 128],
                            xks[k // 4][:, (k % 4) * 512:(k % 4 + 1) * 512],
                            start=(k == 0), stop=(k == KCH - 1))
                    if kind == "v":
                        nc.any.tensor_copy(v2[:, t0:t0 + 512], ps[:])
                        continue
                    # RoPE: g = raw*cos + shift(raw)*sin_signed
                    g2 = q2 if kind == "q" else k2
                    dst = g2[:, t0:t0 + 512]
                    nc.any.tensor_copy(dst, ps[:])               # raw evict
                    sh = work.tile([128, 512], F32R, tag="sh", name=f"sh_{kind}_{b}_{j}")
                    for (d0, s0) in ((0, 32), (32, 0), (64, 96), (96, 64)):
                        nc.sync.dma_start(sh[d0:d0 + 32, :], g2[s0:s0 + 32, t0:t0 + 512])
                    nc.vector.tensor_mul(sh[:], sh[:], sin_sb[:, t0:t0 + 512])
                    nc.vector.tensor_mul(dst, dst, cos_sb[:, t0:t0 + 512])
                    nc.vector.tensor_add(dst, dst, sh[:])

            # ---- P2: transpose V into V' chunks ----
            for i in range(SI):
                tp = psum.tile([128, 128], F32, tag="s0", bufs=2, name=f"tp_{b}_{i}")
                nc.tensor.transpose(tp[:], v2[:, i * 128:(i + 1) * 128], ident_sb[:])
                nc.any.tensor_copy(vp[:, i * 130:i * 130 + 64], tp[:, 0:64])
                nc.any.tensor_copy(vp[:, i * 130 + 65:i * 130 + 129], tp[:, 64:128])

            # ---- P3: attention per t-tile, both heads ----
            for j in range(TJ):
                t0 = j * 512
                ps_y = [psum.tile([65, 512], F32, tag="y", bufs=2, name=f"psy_{b}_{j}_{h}")
                        for h in range(2)]
                n_i = 4 * j + 4
                for i in range(n_i):
                    r = i - 4 * j
                    w0 = max(r, 0) * 128
                    for h in range(2):
                        hp = 64 * h
                        ps_s = psum.tile([128, 512], F32, tag=f"s{h}", bufs=2,
                                         name=f"pss_{b}_{j}_{i}_{h}")
                        nc.tensor.matmul(
                            ps_s[:, w0:512],
                            k2[hp:hp + 64, i * 128:(i + 1) * 128],
                            q2[hp:hp + 64, t0 + w0:t0 + 512],
                            start=True, stop=True)
                        pt = work.tile([128, 512], F32R, tag=f"p{h}",
                                       name=f"pt_{b}_{j}_{i}_{h}")
                        if r >= 0:
                            nc.scalar.activation(pt[:, w0:w0 + 128], ps_s[:, w0:w0 + 128], AF.Exp)
                            nc.vector.tensor_mul(pt[:, w0:w0 + 128], pt[:, w0:w0 + 128], mask_sb[:])
                            if w0 + 128 < 512:
                                nc.scalar.activation(pt[:, w0 + 128:512], ps_s[:, w0 + 128:512], AF.Exp)
                        else:
                            nc.scalar.activation(pt[:], ps_s[:], AF.Exp)
                        vcol = i * 130 + 65 * h
                        nc.tensor.matmul(
                            ps_y[h][:, w0:512],
                            vp[:, vcol:vcol + 65],
                            pt[:, w0:512],
                            start=(i == 0), stop=(i == n_i - 1))
                for h in range(2):
                    nc.any.tensor_copy(yt[64 * h:64 * h + 64, t0:t0 + 512], ps_y[h][0:64, :])
                    nc.any.tensor_copy(rn[32 * h:32 * h + 1, t0:t0 + 512], ps_y[h][64:65, :])

            # ---- P4: normalize + project ----
            # bring the h1 sums row (staged at partition 32) adjacent to h0's
            nc.sync.dma_start(rn[1:2, 0:T], rn[32:33, 0:T])
            # r0 = 1/s into columns [T, 2T)
            with nc.allow_low_precision(reason="f32r annotation; values stay fp32"):
                nc.vector.reciprocal(rn[0:2, T:2 * T], rn[0:2, 0:T])
            # one Newton step: r = r0 * (2 - s*r0); s (cols 0:T) is dead after
            with nc.allow_low_precision(reason="f32r annotation; values stay fp32"):
                nc.vector.tensor_mul(rn[0:2, 0:T], rn[0:2, 0:T], rn[0:2, T:2 * T])
                nc.vector.tensor_scalar(rn[0:2, 0:T], rn[0:2, 0:T], -1.0, 2.0,
                                        op0=mybir.AluOpType.mult,
                                        op1=mybir.AluOpType.add)
                rsb = rn[0:2, T:2 * T]
                nc.vector.tensor_mul(rsb, rsb, rn[0:2, 0:T])
            for j in range(TJ):
                t0 = j * 512
                bc = psum.tile([128, 512], F32, tag="y", bufs=2, name=f"bc_{b}_{j}")
                nc.tensor.matmul(bc[:], sel_sb[:], rsb[:, t0:t0 + 512],
                                 start=True, stop=True)
                nc.vector.tensor_mul(yt[:, t0:t0 + 512], yt[:, t0:t0 + 512], bc[:])
            for blk in range(T // 128):
                g0 = b * T + blk * 128
                for half in range(2):
                    pp = psum.tile([128, 512], F32, tag="y", bufs=2,
                                   name=f"pp_{b}_{blk}_{half}")
                    nc.tensor.matmul(
                        pp[:], yt[:, blk * 128:(blk + 1) * 128],
                        wp_sb[:, half * 512:(half + 1) * 512],
                        start=True, stop=True)
                    ob = work.tile([128, 512], F32, tag="ob", name=f"ob_{b}_{blk}_{half}")
                    nc.any.tensor_copy(ob[:], pp[:])
                    nc.sync.dma_start(out[g0:g0 + 128, half * 512:(half + 1) * 512], ob[:])

    nc.compile()
    return nc


def host_inputs(x, W_qkv, W_proj):
    """Build the per-core input maps (all hardcoded shapes)."""
    x = np.asarray(x, dtype=np.float32)
    W_qkv = np.asarray(W_qkv, dtype=np.float32)
    W_proj = np.asarray(W_proj, dtype=np.float32)

    xT = np.ascontiguousarray(x.reshape(NT, C).T)

    # RoPE tables, matching reference._rope_cos_sin
    inv_freq = 1.0 / (10000.0 ** (np.arange(0, D, 2, dtype=np.float32) / D))
    t = np.arange(T, dtype=np.float32)
    freqs = np.outer(t, inv_freq).astype(np.float32)          # [T, 32]
    emb = np.concatenate([freqs, freqs], axis=-1)             # [T, 64]
    cosT = np.cos(emb).T.astype(np.float32)                   # [64, T]
    sinT = np.sin(emb).T.astype(np.float32)                   # [64, T]
    sin_signed = sinT.copy()
    sin_signed[:32] *= -1.0
    cos2 = np.ascontiguousarray(np.tile(cosT, (2, 1)))        # [128, T]
    sin2 = np.ascontiguousarray(np.tile(sin_signed, (2, 1)))  # [128, T]

    f = np.arange(128, dtype=np.float32)
    mask = (f[None, :] >= f[:, None]).astype(np.float32)      # [s,t]: t>=s
    ident = np.eye(128, dtype=np.float32)
    sel = np.zeros((2, 128), dtype=np.float32)
    sel[0, :64] = 1.0
    sel[1, 64:] = 1.0
    ones = np.ones((128, 1), dtype=np.float32)

    in_maps = []
    for c in range(N_CORES):
        h0, h1 = 2 * c, 2 * c + 1

        def wslice(off, scale=1.0):
            w = np.concatenate([W_qkv[off + D * h0: off + D * h0 + D],
                                W_qkv[off + D * h1: off + D * h1 + D]], axis=0)
            return np.ascontiguousarray(w.T * scale)

        in_maps.append({
            "xT": xT,
            "wq": wslice(0, 0.125),
            "wk": wslice(C),
            "wv": wslice(2 * C),
            "wp": np.ascontiguousarray(W_proj[:, 128 * c:128 * (c + 1)].T),
            "cos2": cos2, "sin2": sin2, "mask": mask,
            "ident": ident, "sel": sel, "ones": ones,
        })
    return in_maps


_NC_CACHE = []


def kernel(x, W_qkv, W_proj):
    from concourse.bass_utils import run_bass_kernel_spmd
    if not _NC_CACHE:
        _NC_CACHE.append(build_nc())
    nc = _NC_CACHE[0]
    in_maps = host_inputs(x, W_qkv, W_proj)
    res = run_bass_kernel_spmd(nc, in_maps, core_ids=list(range(N_CORES)))
    total = np.zeros((NT, C), dtype=np.float32)
    for r in res.results:
        total += r["out_part"]
    return total.reshape(B, T, C)
